# revision 6
# baseline (speedup 1.0000x reference)
"""AnomalyAttention Trainium2 kernel — 8 NeuronCores, batch-sharded.

Math (per batch element b, one per core):
  Q = x@Wq, K = x@Wk, V = x@Wv, sigma = f(x@Ws)
  scores = Q@K^T/32 ; E = exp(scores) ; sumE = AllReduce_b(E)   <- softmax over batch dim
  S = E/sumE ; Z = S@V
  P = inv_norm * exp(-0.5 (dist/sigma)^2) / total               <- fully local

Layout trick: host passes x[b]^T (d-major). With TensorE's out = lhsT.T @ rhs:
  QT[j,n] = (lhsT=Wq[d,j]).T @ (rhs=xT[d,n])
  KT[j,m] = (lhsT=Wk).T @ xT
  V[m,d]  = (lhsT=xT[d,m]).T @ (rhs=Wv[d,d'])
  ST[m,n] = (lhsT=KT[j,m]).T @ (rhs=QT[j,n])     (scores^T; batch-softmax is
                                                  elementwise across cores, so
                                                  orientation is free)
  Z[n,d]  = (lhsT=ST[m,n]).T @ (rhs=V[m,d])
No on-chip transposes anywhere.

v2: the AllReduce is split into two n-halves; each half's S-chain + Z matmuls
pipeline against the other half's collective. V/sigma/prior fill the first
collective's window.
"""

import functools
import math
import sys

sys.path.insert(0, "/opt/trn_rl_repo")

import numpy as np
import ml_dtypes

import concourse.bass as bass
import concourse.bacc as bacc
import concourse.mybir as mybir
import concourse.tile as tile
from concourse.bass_utils import run_bass_kernel_spmd

B, N, D = 8, 1024, 1024
P = 128          # SBUF partitions
NT = N // P      # 8 chunks
FD = 512         # matmul free-dim tile (one PSUM bank of fp32)
NF = N // FD     # 2 free-dim slices ("halves")

BF = mybir.dt.bfloat16
F32 = mybir.dt.float32

INV_SQRT_D = 1.0 / math.sqrt(D)      # 1/32
LN3 = math.log(3.0)
INV_SQRT_2PI = 1.0 / math.sqrt(2.0 * math.pi)


def build_nc():
    nc = bacc.Bacc("TRN2", target_bir_lowering=False, debug=False, num_devices=B)

    xT = nc.dram_tensor("xT", [D, N], BF, kind="ExternalInput").ap()
    Wq = nc.dram_tensor("Wq", [D, D], BF, kind="ExternalInput").ap()
    Wk = nc.dram_tensor("Wk", [D, D], BF, kind="ExternalInput").ap()
    Wv = nc.dram_tensor("Wv", [D, D], BF, kind="ExternalInput").ap()
    Wsr = nc.dram_tensor("Wsr", [P, NT], BF, kind="ExternalInput").ap()   # Ws re-laid [p, chunk]
    d2 = nc.dram_tensor("d2", [N, N], F32, kind="ExternalInput").ap()    # (i-j)^2
    out_z = nc.dram_tensor("out_z", [N, D], F32, kind="ExternalOutput").ap()
    out_p = nc.dram_tensor("out_p", [N, N], F32, kind="ExternalOutput").ap()

    with tile.TileContext(nc) as tc:
        with (
            tc.tile_pool(name="const", bufs=1) as cp,
            tc.tile_pool(name="w", bufs=2) as wp,
            tc.tile_pool(name="big", bufs=1) as bigp,
            tc.tile_pool(name="stage", bufs=2) as stp,
            tc.tile_pool(name="pscr", bufs=3) as pscrp,
            tc.tile_pool(name="zst", bufs=3) as zstp,
            tc.tile_pool(name="ps", bufs=4, space="PSUM") as psp,
            tc.tile_pool(name="ps_small", bufs=2, space="PSUM") as pssp,
            tc.tile_pool(name="dram", bufs=1, space="DRAM") as dramp,
        ):
            # ---------- persistent SBUF ----------
            xT_sb = bigp.tile([P, NT * N], BF, tag="xT")    # chunk k at cols [k*N, (k+1)*N)
            QT_sb = bigp.tile([P, NT * N], BF, tag="QT")
            KT_sb = bigp.tile([P, NT * N], BF, tag="KT")
            V_sb = bigp.tile([P, NT * D], BF, tag="V")
            E_sb = bigp.tile([P, NT * N], BF, tag="E")      # exp(scores^T)
            G_sb = bigp.tile([P, NT * N], BF, tag="G")      # unnormalized gaussian
            ST_sb = bigp.tile([P, NT * N], BF, tag="ST")    # softmax^T

            Ws_sb = cp.tile([P, NT], BF, tag="Ws")
            sraw_sb = cp.tile([1, N], F32, tag="sraw")
            sigc_sb = cp.tile([P, NT], F32, tag="sigc")     # x@Ws, [p, chunk] layout
            u_sb = cp.tile([P, NT], F32, tag="u")
            sg_sb = cp.tile([P, NT], F32, tag="sg")
            e3_sb = cp.tile([P, NT], F32, tag="e3")
            sigma_sb = cp.tile([P, NT], F32, tag="sigma")
            rs_sb = cp.tile([P, NT], F32, tag="rs")         # 1/sigma
            rs2_sb = cp.tile([P, NT], F32, tag="rs2")       # 1/sigma^2
            t_sb = cp.tile([P, NT], F32, tag="t")           # -0.5/sigma^2
            inorm_sb = cp.tile([P, NT], F32, tag="inorm")   # 1/(sqrt(2pi) sigma)
            grs_sb = cp.tile([P, NT], F32, tag="grs")       # gaussian row sums
            rssc_sb = cp.tile([P, NT], F32, tag="rssc")
            rowtot_sb = cp.tile([P, 1], F32, tag="rowtot")
            toti_sb = cp.tile([1, 1], F32, tag="toti")
            totb_sb = cp.tile([P, 1], F32, tag="totb")
            f_sb = cp.tile([P, NT], F32, tag="f")           # inorm/total

            # DRAM bounce buffers: one pair per n-half collective
            cc_in = [dramp.tile([N, FD], BF, name=f"cc_in{h}", tag=f"cc_in{h}")
                     for h in range(NF)]
            cc_out = [dramp.tile([N, FD], BF, addr_space="Shared",
                                 name=f"cc_out{h}", tag=f"cc_out{h}")
                      for h in range(NF)]

            # ---------- input loads (xT/Wq interleaved so matmuls start early) ----------
            wq_t = wp.tile([P, NT * D], BF, tag="w")
            for k in range(NT):
                nc.sync.dma_start(xT_sb[:, k * N:(k + 1) * N], xT[k * P:(k + 1) * P, :])
                nc.sync.dma_start(wq_t[:, k * D:(k + 1) * D], Wq[k * P:(k + 1) * P, :])
            wk_t = wp.tile([P, NT * D], BF, tag="w")
            for k in range(NT):
                nc.sync.dma_start(wk_t[:, k * D:(k + 1) * D], Wk[k * P:(k + 1) * P, :])
            nc.sync.dma_start(Ws_sb[:], Wsr[:])

            def mm_accum(ps, lhs_fn, rhs_fn):
                for k in range(NT):
                    nc.tensor.matmul(
                        ps[:], lhsT=lhs_fn(k), rhs=rhs_fn(k),
                        start=(k == 0), stop=(k == NT - 1),
                    )

            # ---------- projections QT, KT (psum copies on DVE) ----------
            for w_t, o_sb in ((wq_t, QT_sb), (wk_t, KT_sb)):
                for mi in range(NT):
                    for ns in range(NF):
                        ps = psp.tile([P, FD], F32, tag="mm")
                        mm_accum(
                            ps,
                            lambda k, mi=mi, w_t=w_t: w_t[:, k * D + mi * P: k * D + mi * P + P],
                            lambda k, ns=ns: xT_sb[:, k * N + ns * FD: k * N + (ns + 1) * FD],
                        )
                        nc.vector.tensor_copy(
                            o_sb[:, mi * N + ns * FD: mi * N + (ns + 1) * FD], ps[:]
                        )
                if w_t is wq_t:
                    # Wq is dead after this; let Wv reuse its slot and start its DMA now
                    wv_t = wp.tile([P, NT * D], BF, tag="w")
                    for k in range(NT):
                        nc.sync.dma_start(wv_t[:, k * D:(k + 1) * D],
                                          Wv[k * P:(k + 1) * P, :])

            # ---------- sigma matvec + scalar chain (tiny; feeds the prior) ----------
            for ns in range(NF):
                ps = pssp.tile([1, FD], F32, tag="sg")
                mm_accum(
                    ps,
                    lambda k: Ws_sb[:, k:k + 1],
                    lambda k, ns=ns: xT_sb[:, k * N + ns * FD: k * N + (ns + 1) * FD],
                )
                nc.scalar.copy(sraw_sb[:, ns * FD:(ns + 1) * FD], ps[:])
            # [1, N] -> [P, NT] cross-partition move via a DRAM bounce
            sig_scr = dramp.tile([1, N], F32)
            nc.sync.dma_start(sig_scr[:], sraw_sb[:])
            for c in range(NT):
                nc.sync.dma_start(
                    sigc_sb[:, c:c + 1],
                    sig_scr[0:1, c * P:(c + 1) * P].rearrange("o p -> p o"),
                )
            # sigma = 3^(sigmoid(5z) + 1e-5) - 1
            nc.scalar.activation(u_sb[:], sigc_sb[:], mybir.ActivationFunctionType.Exp,
                                 scale=-5.0)                       # exp(-5z)
            nc.vector.tensor_scalar_add(u_sb[:], u_sb[:], 1.0)     # 1 + exp(-5z)
            nc.vector.reciprocal(sg_sb[:], u_sb[:])                # sigmoid(5z)
            nc.vector.tensor_scalar_add(sg_sb[:], sg_sb[:], 1e-5)
            nc.scalar.activation(e3_sb[:], sg_sb[:], mybir.ActivationFunctionType.Exp,
                                 scale=LN3)                        # 3^s
            nc.vector.tensor_scalar_add(sigma_sb[:], e3_sb[:], -1.0)
            nc.vector.reciprocal(rs_sb[:], sigma_sb[:])            # 1/sigma
            nc.vector.tensor_mul(rs2_sb[:], rs_sb[:], rs_sb[:])    # 1/sigma^2
            nc.vector.tensor_scalar_mul(t_sb[:], rs2_sb[:], -0.5)
            nc.vector.tensor_scalar_mul(inorm_sb[:], rs_sb[:], INV_SQRT_2PI)

            # ---------- scores^T -> E, one n-half at a time; AllReduce per half ----
            for ns in range(NF):
                for mi in range(NT):
                    ps = psp.tile([P, FD], F32, tag="mm")
                    mm_accum(
                        ps,
                        lambda k, mi=mi: KT_sb[:, k * N + mi * P: k * N + mi * P + P],
                        lambda k, ns=ns: QT_sb[:, k * N + ns * FD: k * N + (ns + 1) * FD],
                    )
                    e_slice = E_sb[:, mi * N + ns * FD: mi * N + (ns + 1) * FD]
                    nc.scalar.activation(
                        e_slice, ps[:], mybir.ActivationFunctionType.Exp,
                        scale=INV_SQRT_D,
                    )
                    nc.sync.dma_start(cc_in[ns][mi * P:(mi + 1) * P, :], e_slice)
                nc.gpsimd.collective_compute(
                    "AllReduce", mybir.AluOpType.add,
                    replica_groups=[list(range(B))],
                    ins=[cc_in[ns].opt()], outs=[cc_out[ns].opt()],
                )

            # ---------- overlap region: V projection ----------
            for mi in range(NT):
                for ds in range(NF):
                    ps = psp.tile([P, FD], F32, tag="mm")
                    mm_accum(
                        ps,
                        lambda k, mi=mi: xT_sb[:, k * N + mi * P: k * N + mi * P + P],
                        lambda k, ds=ds: wv_t[:, k * D + ds * FD: k * D + (ds + 1) * FD],
                    )
                    nc.scalar.copy(V_sb[:, mi * D + ds * FD: mi * D + (ds + 1) * FD], ps[:])

            # ---------- overlap region: gaussian prior ----------
            for i in range(NT):
                d2_st = pscrp.tile([P, N], F32, tag="pscr")
                nc.sync.dma_start(d2_st[:], d2[i * P:(i + 1) * P, :])
                nc.scalar.activation(
                    G_sb[:, i * N:(i + 1) * N], d2_st[:],
                    mybir.ActivationFunctionType.Exp,
                    scale=t_sb[:, i:i + 1],
                    accum_out=grs_sb[:, i:i + 1],
                )
            # total = sum(inorm * row_sums); P = G * (inorm/total)
            nc.vector.tensor_mul(rssc_sb[:], grs_sb[:], inorm_sb[:])
            nc.vector.reduce_sum(rowtot_sb[:], rssc_sb[:], axis=mybir.AxisListType.X)
            ps_tot = pssp.tile([1, 1], F32, tag="tot")
            ones_col = cp.tile([P, 1], F32, tag="ones_col")
            nc.vector.memset(ones_col[:], 1.0)
            nc.tensor.matmul(ps_tot[:], lhsT=rowtot_sb[:], rhs=ones_col[:],
                             start=True, stop=True)
            nc.vector.reciprocal(toti_sb[:], ps_tot[:])
            # broadcast [1,1] -> [P,1] via a K=1 matmul against a ones row
            ones_row = cp.tile([1, P], F32, tag="ones_row")
            nc.vector.memset(ones_row[:], 1.0)
            ps_b = pssp.tile([P, 1], F32, tag="tot")
            nc.tensor.matmul(ps_b[:], lhsT=ones_row[:], rhs=toti_sb[:],
                             start=True, stop=True)
            nc.vector.tensor_copy(totb_sb[:], ps_b[:])
            nc.vector.tensor_scalar_mul(f_sb[:], inorm_sb[:], totb_sb[:])
            for i in range(NT):
                p_st = pscrp.tile([P, N], F32, tag="pscr")
                nc.vector.tensor_scalar_mul(p_st[:], G_sb[:, i * N:(i + 1) * N],
                                            f_sb[:, i:i + 1])
                nc.sync.dma_start(out_p[i * P:(i + 1) * P, :], p_st[:])

            # ---------- per half: S^T = E/sumE, then Z for that half ----------
            for ns in range(NF):
                for k in range(NT):
                    se_bf = stp.tile([P, FD], BF, tag="sebf")
                    nc.sync.dma_start(se_bf[:], cc_out[ns][k * P:(k + 1) * P, :])
                    se_f = stp.tile([P, FD], F32, tag="sef")
                    nc.vector.tensor_copy(se_f[:], se_bf[:])
                    rcp_f = stp.tile([P, FD], F32, tag="rcpf")
                    nc.vector.reciprocal_approx_fast(rcp_f[:], se_f[:])
                    rcp_b = stp.tile([P, FD], BF, tag="rcpb")
                    nc.vector.tensor_copy(rcp_b[:], rcp_f[:])
                    # numerator multiply on GpSimd to keep DVE free
                    nc.gpsimd.tensor_mul(
                        ST_sb[:, k * N + ns * FD: k * N + (ns + 1) * FD],
                        E_sb[:, k * N + ns * FD: k * N + (ns + 1) * FD],
                        rcp_b[:],
                    )
                for ni in range(ns * NT // NF, (ns + 1) * NT // NF):
                    for ds in range(NF):
                        ps = psp.tile([P, FD], F32, tag="mm")
                        mm_accum(
                            ps,
                            lambda k, ni=ni: ST_sb[:, k * N + ni * P: k * N + ni * P + P],
                            lambda k, ds=ds: V_sb[:, k * D + ds * FD: k * D + (ds + 1) * FD],
                        )
                        z_st = zstp.tile([P, FD], F32, tag="z")
                        nc.scalar.copy(z_st[:], ps[:])
                        nc.sync.dma_start(
                            out_z[ni * P:(ni + 1) * P, ds * FD:(ds + 1) * FD], z_st[:]
                        )

    nc.compile()
    return nc


@functools.cache
def _get_nc():
    return build_nc()


def _make_in_maps(x, Wq, Wk, Wv, Ws):
    bf = ml_dtypes.bfloat16
    idx = np.arange(N, dtype=np.float32)
    d2 = np.square(idx[:, None] - idx[None, :])  # exact in fp32
    wq = np.asarray(Wq, np.float32).astype(bf)
    wk = np.asarray(Wk, np.float32).astype(bf)
    wv = np.asarray(Wv, np.float32).astype(bf)
    wsr = np.ascontiguousarray(
        np.asarray(Ws, np.float32)[:, 0].reshape(NT, P).T
    ).astype(bf)
    in_maps = []
    for b in range(B):
        xTb = np.ascontiguousarray(np.asarray(x[b], np.float32).T).astype(bf)
        in_maps.append(
            {"xT": xTb, "Wq": wq, "Wk": wk, "Wv": wv, "Wsr": wsr, "d2": d2}
        )
    return in_maps


def run(x, Wq, Wk, Wv, Ws, trace=False):
    nc = _get_nc()
    in_maps = _make_in_maps(x, Wq, Wk, Wv, Ws)
    res = run_bass_kernel_spmd(nc, in_maps, core_ids=list(range(B)), trace=trace)
    Z = np.stack([res.results[b]["out_z"] for b in range(B)])
    Pp = np.stack([res.results[b]["out_p"] for b in range(B)])
    return (Z, Pp), res


def kernel(x, Wq, Wk, Wv, Ws):
    for _ in range(2):
        (Z, Pp), _ = run(x, Wq, Wk, Wv, Ws, trace=False)
        if np.isfinite(Z).all() and np.isfinite(Pp).all():
            break
    return Z, Pp


# revision 7
# speedup vs baseline: 1.1504x; 1.1504x over previous
"""AnomalyAttention Trainium2 kernel — 8 NeuronCores, batch-sharded.

Math (per batch element b, one per core):
  scores = (x Wq)(x Wk)^T/32 = x W2 x^T /32   with W2 = Wq@Wk^T precomputed on host
  E = exp(scores) ; sumE = AllReduce_b(E)     <- softmax over batch dim
  S = E/sumE ; Z = S@(x Wv)
  P = inv_norm * exp(-0.5 (dist/sigma)^2) / total    <- fully local prior

Layout trick: host passes x[b]^T (d-major). With TensorE's out = lhsT.T @ rhs:
  AT[e,n] = (lhsT=W2[d,e]).T @ (rhs=xT[d,n])         (A = x@W2)
  ST[m,n] = (lhsT=xT[e,m]).T @ (rhs=AT[e,n])         (= scores^T)
  V[m,d]  = (lhsT=xT[d,m]).T @ (rhs=Wv[d,d'])
  Z[n,d]  = (lhsT=S^T[m,n]).T @ (rhs=V[m,d])
4 big matmuls, no on-chip transposes.

The AllReduce is split into two n-halves, each issued as soon as its half of
E is on DRAM; S-chain (recip+mul) is pipelined across ScalarE/VectorE/GpSimd;
V + the gaussian prior fill the collective windows.
"""

import functools
import math
import sys

sys.path.insert(0, "/opt/trn_rl_repo")

import numpy as np
import ml_dtypes

import concourse.bass as bass
import concourse.bacc as bacc
import concourse.mybir as mybir
import concourse.tile as tile
from concourse.bass_utils import run_bass_kernel_spmd

B, N, D = 8, 1024, 1024
P = 128          # SBUF partitions
NT = N // P      # 8 chunks
FD = 512         # matmul free-dim tile (one PSUM bank of fp32)
NF = N // FD     # 2 free-dim slices ("halves")

BF = mybir.dt.bfloat16
F32 = mybir.dt.float32

INV_SQRT_D = 1.0 / math.sqrt(D)      # 1/32
LN3 = math.log(3.0)
INV_SQRT_2PI = 1.0 / math.sqrt(2.0 * math.pi)


def build_nc():
    nc = bacc.Bacc("TRN2", target_bir_lowering=False, debug=False, num_devices=B)

    xT = nc.dram_tensor("xT", [D, N], BF, kind="ExternalInput").ap()
    W2 = nc.dram_tensor("W2", [D, D], BF, kind="ExternalInput").ap()
    Wv = nc.dram_tensor("Wv", [D, D], BF, kind="ExternalInput").ap()
    Wsr = nc.dram_tensor("Wsr", [P, NT], BF, kind="ExternalInput").ap()   # Ws re-laid [p, chunk]
    d2 = nc.dram_tensor("d2", [N, N], F32, kind="ExternalInput").ap()    # (i-j)^2
    out_z = nc.dram_tensor("out_z", [N, D], F32, kind="ExternalOutput").ap()
    out_p = nc.dram_tensor("out_p", [N, N], F32, kind="ExternalOutput").ap()

    with tile.TileContext(nc) as tc:
        with (
            tc.tile_pool(name="const", bufs=1) as cp,
            tc.tile_pool(name="w", bufs=2) as wp,
            tc.tile_pool(name="big", bufs=1) as bigp,
            tc.tile_pool(name="stage", bufs=3) as stp,
            tc.tile_pool(name="pscr", bufs=3) as pscrp,
            tc.tile_pool(name="zst", bufs=3) as zstp,
            tc.tile_pool(name="ps", bufs=4, space="PSUM") as psp,
            tc.tile_pool(name="ps_small", bufs=2, space="PSUM") as pssp,
            tc.tile_pool(name="dram", bufs=1, space="DRAM") as dramp,
        ):
            # ---------- persistent SBUF ----------
            xT_sb = bigp.tile([P, NT * N], BF, tag="xT")    # chunk k at cols [k*N, (k+1)*N)
            AT_sb = bigp.tile([P, NT * N], BF, tag="AT")    # (x@W2)^T
            V_sb = bigp.tile([P, NT * D], BF, tag="V")
            E_sb = bigp.tile([P, NT * N], BF, tag="E")      # exp(scores^T)
            G_sb = bigp.tile([P, NT * N], BF, tag="G")      # unnormalized gaussian
            ST_sb = bigp.tile([P, NT * N], BF, tag="ST")    # softmax^T

            Ws_sb = cp.tile([P, NT], BF, tag="Ws")
            sraw_sb = cp.tile([1, N], F32, tag="sraw")
            sigc_sb = cp.tile([P, NT], F32, tag="sigc")     # x@Ws, [p, chunk] layout
            u_sb = cp.tile([P, NT], F32, tag="u")
            sg_sb = cp.tile([P, NT], F32, tag="sg")
            e3_sb = cp.tile([P, NT], F32, tag="e3")
            sigma_sb = cp.tile([P, NT], F32, tag="sigma")
            rs_sb = cp.tile([P, NT], F32, tag="rs")         # 1/sigma
            rs2_sb = cp.tile([P, NT], F32, tag="rs2")       # 1/sigma^2
            t_sb = cp.tile([P, NT], F32, tag="t")           # -0.5/sigma^2
            inorm_sb = cp.tile([P, NT], F32, tag="inorm")   # 1/(sqrt(2pi) sigma)
            grs_sb = cp.tile([P, NT], F32, tag="grs")       # gaussian row sums
            rssc_sb = cp.tile([P, NT], F32, tag="rssc")
            rowtot_sb = cp.tile([P, 1], F32, tag="rowtot")
            toti_sb = cp.tile([1, 1], F32, tag="toti")
            totb_sb = cp.tile([P, 1], F32, tag="totb")
            f_sb = cp.tile([P, NT], F32, tag="f")           # inorm/total

            # DRAM bounce buffers: one pair per n-half collective
            cc_in = [dramp.tile([N, FD], BF, name=f"cc_in{h}", tag=f"cc_in{h}")
                     for h in range(NF)]
            cc_out = [dramp.tile([N, FD], BF, addr_space="Shared",
                                 name=f"cc_out{h}", tag=f"cc_out{h}")
                      for h in range(NF)]

            # ---------- input loads ----------
            # first chunk of xT and W2 split into 32-partition strips so the
            # first matmul's inputs land fast (parallel DMA queues)
            w2_t = wp.tile([P, NT * D], BF, tag="w")
            for p0 in range(0, P, 32):
                nc.sync.dma_start(xT_sb[p0:p0 + 32, 0:N], xT[p0:p0 + 32, :])
                nc.sync.dma_start(w2_t[p0:p0 + 32, 0:D], W2[p0:p0 + 32, :])
            for k in range(1, NT):
                nc.sync.dma_start(xT_sb[:, k * N:(k + 1) * N], xT[k * P:(k + 1) * P, :])
                nc.sync.dma_start(w2_t[:, k * D:(k + 1) * D], W2[k * P:(k + 1) * P, :])
            nc.sync.dma_start(Ws_sb[:], Wsr[:])

            def mm_accum(ps, lhs_fn, rhs_fn):
                for k in range(NT):
                    nc.tensor.matmul(
                        ps[:], lhsT=lhs_fn(k), rhs=rhs_fn(k),
                        start=(k == 0), stop=(k == NT - 1),
                    )

            # ---------- AT = (x@W2)^T, half ns at a time ----------
            for ns in range(NF):
                for mi in range(NT):
                    ps = psp.tile([P, FD], F32, tag="mm")
                    mm_accum(
                        ps,
                        lambda k, mi=mi: w2_t[:, k * D + mi * P: k * D + mi * P + P],
                        lambda k, ns=ns: xT_sb[:, k * N + ns * FD: k * N + (ns + 1) * FD],
                    )
                    nc.vector.tensor_copy(
                        AT_sb[:, mi * N + ns * FD: mi * N + (ns + 1) * FD], ps[:]
                    )
                if ns == 0:
                    # W2 dead after ns=1 issues; Wv slot + DMA early enough for V
                    wv_t = wp.tile([P, NT * D], BF, tag="w")

            # ---------- scores^T -> E, one n-half at a time; AllReduce per half ----
            for ns in range(NF):
                for mi in range(NT):
                    ps = psp.tile([P, FD], F32, tag="mm")
                    mm_accum(
                        ps,
                        lambda k, mi=mi: xT_sb[:, k * N + mi * P: k * N + mi * P + P],
                        lambda k, ns=ns: AT_sb[:, k * N + ns * FD: k * N + (ns + 1) * FD],
                    )
                    e_slice = E_sb[:, mi * N + ns * FD: mi * N + (ns + 1) * FD]
                    nc.scalar.activation(
                        e_slice, ps[:], mybir.ActivationFunctionType.Exp,
                        scale=INV_SQRT_D,
                    )
                    nc.sync.dma_start(cc_in[ns][mi * P:(mi + 1) * P, :], e_slice)
                nc.gpsimd.collective_compute(
                    "AllReduce", mybir.AluOpType.add,
                    replica_groups=[list(range(B))],
                    ins=[cc_in[ns].opt()], outs=[cc_out[ns].opt()],
                )

            # Wv load (after the scores phase's xT/AT reads are in flight)
            for k in range(NT):
                nc.sync.dma_start(wv_t[:, k * D:(k + 1) * D], Wv[k * P:(k + 1) * P, :])

            # ---------- sigma matvec + scalar chain (tiny; feeds the prior) ----------
            for ns in range(NF):
                ps = pssp.tile([1, FD], F32, tag="sg")
                mm_accum(
                    ps,
                    lambda k: Ws_sb[:, k:k + 1],
                    lambda k, ns=ns: xT_sb[:, k * N + ns * FD: k * N + (ns + 1) * FD],
                )
                nc.scalar.copy(sraw_sb[:, ns * FD:(ns + 1) * FD], ps[:])
            # [1, N] -> [P, NT] cross-partition move via a DRAM bounce
            sig_scr = dramp.tile([1, N], F32)
            nc.sync.dma_start(sig_scr[:], sraw_sb[:])
            for c in range(NT):
                nc.sync.dma_start(
                    sigc_sb[:, c:c + 1],
                    sig_scr[0:1, c * P:(c + 1) * P].rearrange("o p -> p o"),
                )
            # sigma = 3^(sigmoid(5z) + 1e-5) - 1
            nc.scalar.activation(u_sb[:], sigc_sb[:], mybir.ActivationFunctionType.Exp,
                                 scale=-5.0)                       # exp(-5z)
            nc.vector.tensor_scalar_add(u_sb[:], u_sb[:], 1.0)     # 1 + exp(-5z)
            nc.vector.reciprocal(sg_sb[:], u_sb[:])                # sigmoid(5z)
            nc.vector.tensor_scalar_add(sg_sb[:], sg_sb[:], 1e-5)
            nc.scalar.activation(e3_sb[:], sg_sb[:], mybir.ActivationFunctionType.Exp,
                                 scale=LN3)                        # 3^s
            nc.vector.tensor_scalar_add(sigma_sb[:], e3_sb[:], -1.0)
            nc.vector.reciprocal(rs_sb[:], sigma_sb[:])            # 1/sigma
            nc.vector.tensor_mul(rs2_sb[:], rs_sb[:], rs_sb[:])    # 1/sigma^2
            nc.vector.tensor_scalar_mul(t_sb[:], rs2_sb[:], -0.5)
            nc.vector.tensor_scalar_mul(inorm_sb[:], rs_sb[:], INV_SQRT_2PI)

            # ---------- V projection (fills the collective window) ----------
            for mi in range(NT):
                for ds in range(NF):
                    ps = psp.tile([P, FD], F32, tag="mm")
                    mm_accum(
                        ps,
                        lambda k, mi=mi: xT_sb[:, k * N + mi * P: k * N + mi * P + P],
                        lambda k, ds=ds: wv_t[:, k * D + ds * FD: k * D + (ds + 1) * FD],
                    )
                    nc.scalar.copy(V_sb[:, mi * D + ds * FD: mi * D + (ds + 1) * FD], ps[:])

            # ---------- gaussian prior (ACT/DVE work during collectives) ----------
            for i in range(NT):
                d2_st = pscrp.tile([P, N], F32, tag="pscr")
                nc.sync.dma_start(d2_st[:], d2[i * P:(i + 1) * P, :])
                nc.scalar.activation(
                    G_sb[:, i * N:(i + 1) * N], d2_st[:],
                    mybir.ActivationFunctionType.Exp,
                    scale=t_sb[:, i:i + 1],
                    accum_out=grs_sb[:, i:i + 1],
                )
            # total = sum(inorm * row_sums); P = G * (inorm/total)
            nc.vector.tensor_mul(rssc_sb[:], grs_sb[:], inorm_sb[:])
            nc.vector.reduce_sum(rowtot_sb[:], rssc_sb[:], axis=mybir.AxisListType.X)
            ps_tot = pssp.tile([1, 1], F32, tag="tot")
            ones_col = cp.tile([P, 1], F32, tag="ones_col")
            nc.vector.memset(ones_col[:], 1.0)
            nc.tensor.matmul(ps_tot[:], lhsT=rowtot_sb[:], rhs=ones_col[:],
                             start=True, stop=True)
            nc.vector.reciprocal(toti_sb[:], ps_tot[:])
            # broadcast [1,1] -> [P,1] via a K=1 matmul against a ones row
            ones_row = cp.tile([1, P], F32, tag="ones_row")
            nc.vector.memset(ones_row[:], 1.0)
            ps_b = pssp.tile([P, 1], F32, tag="tot")
            nc.tensor.matmul(ps_b[:], lhsT=ones_row[:], rhs=toti_sb[:],
                             start=True, stop=True)
            nc.vector.tensor_copy(totb_sb[:], ps_b[:])
            nc.vector.tensor_scalar_mul(f_sb[:], inorm_sb[:], totb_sb[:])
            for i in range(NT):
                p_st = pscrp.tile([P, N], F32, tag="pscr")
                nc.vector.tensor_scalar_mul(p_st[:], G_sb[:, i * N:(i + 1) * N],
                                            f_sb[:, i:i + 1])
                nc.sync.dma_start(out_p[i * P:(i + 1) * P, :], p_st[:])

            # ---------- per half: S^T = E/sumE (3-engine pipeline), then Z ----------
            for ns in range(NF):
                for k in range(NT):
                    se_bf = stp.tile([P, FD], BF, tag="sebf")
                    nc.sync.dma_start(se_bf[:], cc_out[ns][k * P:(k + 1) * P, :])
                    se_f = stp.tile([P, FD], F32, tag="sef")
                    nc.scalar.copy(se_f[:], se_bf[:])            # ACT: bf16 -> f32
                    rcp_f = stp.tile([P, FD], F32, tag="rcpf")
                    nc.vector.reciprocal_approx_fast(rcp_f[:], se_f[:])   # DVE
                    rcp_b = stp.tile([P, FD], BF, tag="rcpb")
                    nc.vector.tensor_copy(rcp_b[:], rcp_f[:])    # DVE: f32 -> bf16 (2x)
                    nc.gpsimd.tensor_mul(                        # GpSimd: numerator
                        ST_sb[:, k * N + ns * FD: k * N + (ns + 1) * FD],
                        E_sb[:, k * N + ns * FD: k * N + (ns + 1) * FD],
                        rcp_b[:],
                    )
                for ni in range(ns * NT // NF, (ns + 1) * NT // NF):
                    for ds in range(NF):
                        ps = psp.tile([P, FD], F32, tag="mm")
                        mm_accum(
                            ps,
                            lambda k, ni=ni: ST_sb[:, k * N + ni * P: k * N + ni * P + P],
                            lambda k, ds=ds: V_sb[:, k * D + ds * FD: k * D + (ds + 1) * FD],
                        )
                        z_st = zstp.tile([P, FD], F32, tag="z")
                        nc.scalar.copy(z_st[:], ps[:])
                        nc.sync.dma_start(
                            out_z[ni * P:(ni + 1) * P, ds * FD:(ds + 1) * FD], z_st[:]
                        )

    nc.compile()
    return nc


@functools.cache
def _get_nc():
    return build_nc()


def _make_in_maps(x, Wq, Wk, Wv, Ws):
    bf = ml_dtypes.bfloat16
    idx = np.arange(N, dtype=np.float32)
    d2 = np.square(idx[:, None] - idx[None, :])  # exact in fp32
    w2 = (np.asarray(Wq, np.float32) @ np.asarray(Wk, np.float32).T).astype(bf)
    wv = np.asarray(Wv, np.float32).astype(bf)
    wsr = np.ascontiguousarray(
        np.asarray(Ws, np.float32)[:, 0].reshape(NT, P).T
    ).astype(bf)
    in_maps = []
    for b in range(B):
        xTb = np.ascontiguousarray(np.asarray(x[b], np.float32).T).astype(bf)
        in_maps.append(
            {"xT": xTb, "W2": w2, "Wv": wv, "Wsr": wsr, "d2": d2}
        )
    return in_maps


def run(x, Wq, Wk, Wv, Ws, trace=False):
    nc = _get_nc()
    in_maps = _make_in_maps(x, Wq, Wk, Wv, Ws)
    res = run_bass_kernel_spmd(nc, in_maps, core_ids=list(range(B)), trace=trace)
    Z = np.stack([res.results[b]["out_z"] for b in range(B)])
    Pp = np.stack([res.results[b]["out_p"] for b in range(B)])
    return (Z, Pp), res


def kernel(x, Wq, Wk, Wv, Ws):
    for _ in range(2):
        (Z, Pp), _ = run(x, Wq, Wk, Wv, Ws, trace=False)
        if np.isfinite(Z).all() and np.isfinite(Pp).all():
            break
    return Z, Pp


# revision 8
# speedup vs baseline: 1.1897x; 1.0342x over previous
"""AnomalyAttention Trainium2 kernel — 8 NeuronCores, batch-sharded.

Math (per batch element b, one per core):
  scores = (x Wq)(x Wk)^T/32 = x W2 x^T /32   with W2 = Wq@Wk^T precomputed on host
  E = exp(scores) ; sumE = AllReduce_b(E)     <- softmax over batch dim
  S = E/sumE ; Z = S@(x Wv)
  P = inv_norm * exp(-0.5 (dist/sigma)^2) / total    <- fully local prior

Layout trick: host passes x[b]^T (d-major). With TensorE's out = lhsT.T @ rhs:
  AT[e,n] = (lhsT=W2[d,e]).T @ (rhs=xT[d,n])         (A = x@W2)
  ST[m,n] = (lhsT=xT[e,m]).T @ (rhs=AT[e,n])         (= scores^T)
  V[m,d]  = (lhsT=xT[d,m]).T @ (rhs=Wv[d,d'])
  Z[n,d]  = (lhsT=S^T[m,n]).T @ (rhs=V[m,d])
4 big matmuls, no on-chip transposes.

The AllReduce is split into two n-halves, each issued as soon as its half of
E is on DRAM; S-chain (recip+mul) is pipelined across ScalarE/VectorE/GpSimd;
V + the gaussian prior fill the collective windows.
"""

import functools
import math
import sys

sys.path.insert(0, "/opt/trn_rl_repo")

import numpy as np
import ml_dtypes

import concourse.bass as bass
import concourse.bacc as bacc
import concourse.mybir as mybir
import concourse.tile as tile
from concourse.bass_utils import run_bass_kernel_spmd

B, N, D = 8, 1024, 1024
P = 128          # SBUF partitions
NT = N // P      # 8 chunks
FD = 512         # matmul free-dim tile (one PSUM bank of fp32)
NF = N // FD     # 2 free-dim slices ("halves")

BF = mybir.dt.bfloat16
F32 = mybir.dt.float32

INV_SQRT_D = 1.0 / math.sqrt(D)      # 1/32
LN3 = math.log(3.0)
INV_SQRT_2PI = 1.0 / math.sqrt(2.0 * math.pi)


def build_nc():
    nc = bacc.Bacc("TRN2", target_bir_lowering=False, debug=False, num_devices=B)

    xT = nc.dram_tensor("xT", [D, N], BF, kind="ExternalInput").ap()
    W2 = nc.dram_tensor("W2", [D, D], BF, kind="ExternalInput").ap()
    Wv = nc.dram_tensor("Wv", [D, D], BF, kind="ExternalInput").ap()
    Wsr = nc.dram_tensor("Wsr", [P, NT], BF, kind="ExternalInput").ap()   # Ws re-laid [p, chunk]
    d2 = nc.dram_tensor("d2", [N, N], F32, kind="ExternalInput").ap()    # (i-j)^2
    out_z = nc.dram_tensor("out_z", [N, D], F32, kind="ExternalOutput").ap()
    out_p = nc.dram_tensor("out_p", [N, N], F32, kind="ExternalOutput").ap()

    with tile.TileContext(nc) as tc:
        with (
            tc.tile_pool(name="const", bufs=1) as cp,
            tc.tile_pool(name="w", bufs=2) as wp,
            tc.tile_pool(name="big", bufs=1) as bigp,
            tc.tile_pool(name="stage", bufs=3) as stp,
            tc.tile_pool(name="pscr", bufs=3) as pscrp,
            tc.tile_pool(name="zst", bufs=3) as zstp,
            tc.tile_pool(name="ps", bufs=4, space="PSUM") as psp,
            tc.tile_pool(name="ps_small", bufs=2, space="PSUM") as pssp,
            tc.tile_pool(name="dram", bufs=1, space="DRAM") as dramp,
        ):
            # ---------- persistent SBUF ----------
            xT_sb = bigp.tile([P, NT * N], BF, tag="xT")    # chunk k at cols [k*N, (k+1)*N)
            AT_sb = bigp.tile([P, NT * N], BF, tag="AT")    # (x@W2)^T
            V_sb = bigp.tile([P, NT * D], BF, tag="V")
            E_sb = bigp.tile([P, NT * N], BF, tag="E")      # exp(scores^T)
            G_sb = bigp.tile([P, NT * N], BF, tag="G")      # unnormalized gaussian
            ST_sb = bigp.tile([P, NT * N], BF, tag="ST")    # softmax^T

            Ws_sb = cp.tile([P, NT], BF, tag="Ws")
            sraw_sb = cp.tile([1, N], F32, tag="sraw")
            sigc_sb = cp.tile([P, NT], F32, tag="sigc")     # x@Ws, [p, chunk] layout
            u_sb = cp.tile([P, NT], F32, tag="u")
            sg_sb = cp.tile([P, NT], F32, tag="sg")
            e3_sb = cp.tile([P, NT], F32, tag="e3")
            sigma_sb = cp.tile([P, NT], F32, tag="sigma")
            rs_sb = cp.tile([P, NT], F32, tag="rs")         # 1/sigma
            rs2_sb = cp.tile([P, NT], F32, tag="rs2")       # 1/sigma^2
            t_sb = cp.tile([P, NT], F32, tag="t")           # -0.5/sigma^2
            inorm_sb = cp.tile([P, NT], F32, tag="inorm")   # 1/(sqrt(2pi) sigma)
            grs_sb = cp.tile([P, NT], F32, tag="grs")       # gaussian row sums
            rssc_sb = cp.tile([P, NT], F32, tag="rssc")
            rowtot_sb = cp.tile([P, 1], F32, tag="rowtot")
            toti_sb = cp.tile([1, 1], F32, tag="toti")
            totb_sb = cp.tile([P, 1], F32, tag="totb")
            f_sb = cp.tile([P, NT], F32, tag="f")           # inorm/total

            # DRAM bounce buffers: one pair per n-half collective
            cc_in = [dramp.tile([N, FD], BF, name=f"cc_in{h}", tag=f"cc_in{h}")
                     for h in range(NF)]
            cc_out = [dramp.tile([N, FD], BF, addr_space="Shared",
                                 name=f"cc_out{h}", tag=f"cc_out{h}")
                      for h in range(NF)]

            # ---------- input loads ----------
            # first chunk of xT and W2 split into 32-partition strips so the
            # first matmul's inputs land fast (parallel DMA queues)
            w2_t = wp.tile([P, NT * D], BF, tag="w")
            for p0 in range(0, P, 32):
                nc.sync.dma_start(xT_sb[p0:p0 + 32, 0:N], xT[p0:p0 + 32, :])
                nc.sync.dma_start(w2_t[p0:p0 + 32, 0:D], W2[p0:p0 + 32, :])
            for k in range(1, NT):
                nc.sync.dma_start(xT_sb[:, k * N:(k + 1) * N], xT[k * P:(k + 1) * P, :])
                nc.sync.dma_start(w2_t[:, k * D:(k + 1) * D], W2[k * P:(k + 1) * P, :])
            nc.sync.dma_start(Ws_sb[:], Wsr[:])

            def mm_accum(ps, lhs_fn, rhs_fn):
                for k in range(NT):
                    nc.tensor.matmul(
                        ps[:], lhsT=lhs_fn(k), rhs=rhs_fn(k),
                        start=(k == 0), stop=(k == NT - 1),
                    )

            # ---------- warm-up collective: sync cores + ncfw before the real ones
            cc_w_in = dramp.tile([1, 16], F32, name="cc_w_in", tag="cc_w_in")
            cc_w_out = dramp.tile([1, 16], F32, addr_space="Shared",
                                  name="cc_w_out", tag="cc_w_out")
            warm_sb = cp.tile([1, 16], F32, tag="warm_sb")
            nc.vector.memset(warm_sb[:], 1.0)
            nc.sync.dma_start(cc_w_in[:], warm_sb[:])
            nc.gpsimd.collective_compute(
                "AllReduce", mybir.AluOpType.add,
                replica_groups=[list(range(B))],
                ins=[cc_w_in.opt()], outs=[cc_w_out.opt()],
            )

            # ---------- per half: AT = (x@W2)^T, scores^T -> E, AllReduce ----------
            for ns in range(NF):
                for mi in range(NT):
                    ps = psp.tile([P, FD], F32, tag="mm")
                    mm_accum(
                        ps,
                        lambda k, mi=mi: w2_t[:, k * D + mi * P: k * D + mi * P + P],
                        lambda k, ns=ns: xT_sb[:, k * N + ns * FD: k * N + (ns + 1) * FD],
                    )
                    nc.vector.tensor_copy(
                        AT_sb[:, mi * N + ns * FD: mi * N + (ns + 1) * FD], ps[:]
                    )
                for mi in range(NT):
                    ps = psp.tile([P, FD], F32, tag="mm")
                    mm_accum(
                        ps,
                        lambda k, mi=mi: xT_sb[:, k * N + mi * P: k * N + mi * P + P],
                        lambda k, ns=ns: AT_sb[:, k * N + ns * FD: k * N + (ns + 1) * FD],
                    )
                    e_slice = E_sb[:, mi * N + ns * FD: mi * N + (ns + 1) * FD]
                    nc.scalar.activation(
                        e_slice, ps[:], mybir.ActivationFunctionType.Exp,
                        scale=INV_SQRT_D,
                    )
                    nc.sync.dma_start(cc_in[ns][mi * P:(mi + 1) * P, :], e_slice)
                nc.gpsimd.collective_compute(
                    "AllReduce", mybir.AluOpType.add,
                    replica_groups=[list(range(B))],
                    ins=[cc_in[ns].opt()], outs=[cc_out[ns].opt()],
                )
                if ns == 0:
                    # W2 dead after ns=1 issues; Wv slot + DMA early enough for V
                    wv_t = wp.tile([P, NT * D], BF, tag="w")

            # Wv load (after the scores phase's xT/AT reads are in flight)
            for k in range(NT):
                nc.sync.dma_start(wv_t[:, k * D:(k + 1) * D], Wv[k * P:(k + 1) * P, :])

            # ---------- sigma matvec + scalar chain (tiny; feeds the prior) ----------
            for ns in range(NF):
                ps = pssp.tile([1, FD], F32, tag="sg")
                mm_accum(
                    ps,
                    lambda k: Ws_sb[:, k:k + 1],
                    lambda k, ns=ns: xT_sb[:, k * N + ns * FD: k * N + (ns + 1) * FD],
                )
                nc.scalar.copy(sraw_sb[:, ns * FD:(ns + 1) * FD], ps[:])
            # [1, N] -> [P, NT] cross-partition move via a DRAM bounce
            sig_scr = dramp.tile([1, N], F32)
            nc.sync.dma_start(sig_scr[:], sraw_sb[:])
            for c in range(NT):
                nc.sync.dma_start(
                    sigc_sb[:, c:c + 1],
                    sig_scr[0:1, c * P:(c + 1) * P].rearrange("o p -> p o"),
                )
            # sigma = 3^(sigmoid(5z) + 1e-5) - 1
            nc.scalar.activation(u_sb[:], sigc_sb[:], mybir.ActivationFunctionType.Exp,
                                 scale=-5.0)                       # exp(-5z)
            nc.vector.tensor_scalar_add(u_sb[:], u_sb[:], 1.0)     # 1 + exp(-5z)
            nc.vector.reciprocal(sg_sb[:], u_sb[:])                # sigmoid(5z)
            nc.vector.tensor_scalar_add(sg_sb[:], sg_sb[:], 1e-5)
            nc.scalar.activation(e3_sb[:], sg_sb[:], mybir.ActivationFunctionType.Exp,
                                 scale=LN3)                        # 3^s
            nc.vector.tensor_scalar_add(sigma_sb[:], e3_sb[:], -1.0)
            nc.vector.reciprocal(rs_sb[:], sigma_sb[:])            # 1/sigma
            nc.vector.tensor_mul(rs2_sb[:], rs_sb[:], rs_sb[:])    # 1/sigma^2
            nc.vector.tensor_scalar_mul(t_sb[:], rs2_sb[:], -0.5)
            nc.vector.tensor_scalar_mul(inorm_sb[:], rs_sb[:], INV_SQRT_2PI)

            # ---------- V projection (fills the collective window) ----------
            for mi in range(NT):
                for ds in range(NF):
                    ps = psp.tile([P, FD], F32, tag="mm")
                    mm_accum(
                        ps,
                        lambda k, mi=mi: xT_sb[:, k * N + mi * P: k * N + mi * P + P],
                        lambda k, ds=ds: wv_t[:, k * D + ds * FD: k * D + (ds + 1) * FD],
                    )
                    nc.scalar.copy(V_sb[:, mi * D + ds * FD: mi * D + (ds + 1) * FD], ps[:])

            # ---------- gaussian prior (ACT/DVE work during collectives) ----------
            for i in range(NT):
                d2_st = pscrp.tile([P, N], F32, tag="pscr")
                nc.sync.dma_start(d2_st[:], d2[i * P:(i + 1) * P, :])
                nc.scalar.activation(
                    G_sb[:, i * N:(i + 1) * N], d2_st[:],
                    mybir.ActivationFunctionType.Exp,
                    scale=t_sb[:, i:i + 1],
                    accum_out=grs_sb[:, i:i + 1],
                )
            # total = sum(inorm * row_sums); P = G * (inorm/total)
            nc.vector.tensor_mul(rssc_sb[:], grs_sb[:], inorm_sb[:])
            nc.vector.reduce_sum(rowtot_sb[:], rssc_sb[:], axis=mybir.AxisListType.X)
            ps_tot = pssp.tile([1, 1], F32, tag="tot")
            ones_col = cp.tile([P, 1], F32, tag="ones_col")
            nc.vector.memset(ones_col[:], 1.0)
            nc.tensor.matmul(ps_tot[:], lhsT=rowtot_sb[:], rhs=ones_col[:],
                             start=True, stop=True)
            nc.vector.reciprocal(toti_sb[:], ps_tot[:])
            # broadcast [1,1] -> [P,1] via a K=1 matmul against a ones row
            ones_row = cp.tile([1, P], F32, tag="ones_row")
            nc.vector.memset(ones_row[:], 1.0)
            ps_b = pssp.tile([P, 1], F32, tag="tot")
            nc.tensor.matmul(ps_b[:], lhsT=ones_row[:], rhs=toti_sb[:],
                             start=True, stop=True)
            nc.vector.tensor_copy(totb_sb[:], ps_b[:])
            nc.vector.tensor_scalar_mul(f_sb[:], inorm_sb[:], totb_sb[:])
            for i in range(NT):
                p_st = pscrp.tile([P, N], F32, tag="pscr")
                nc.vector.tensor_scalar_mul(p_st[:], G_sb[:, i * N:(i + 1) * N],
                                            f_sb[:, i:i + 1])
                nc.sync.dma_start(out_p[i * P:(i + 1) * P, :], p_st[:])

            # ---------- per half: S^T = E/sumE (3-engine pipeline), then Z ----------
            for ns in range(NF):
                for k in range(NT):
                    se_bf = stp.tile([P, FD], BF, tag="sebf")
                    nc.sync.dma_start(se_bf[:], cc_out[ns][k * P:(k + 1) * P, :])
                    se_f = stp.tile([P, FD], F32, tag="sef")
                    nc.scalar.copy(se_f[:], se_bf[:])            # ACT: bf16 -> f32
                    rcp_f = stp.tile([P, FD], F32, tag="rcpf")
                    nc.vector.reciprocal_approx_fast(rcp_f[:], se_f[:])   # DVE
                    rcp_b = stp.tile([P, FD], BF, tag="rcpb")
                    nc.vector.tensor_copy(rcp_b[:], rcp_f[:])    # DVE: f32 -> bf16 (2x)
                    nc.gpsimd.tensor_mul(                        # GpSimd: numerator
                        ST_sb[:, k * N + ns * FD: k * N + (ns + 1) * FD],
                        E_sb[:, k * N + ns * FD: k * N + (ns + 1) * FD],
                        rcp_b[:],
                    )
                for ni in range(ns * NT // NF, (ns + 1) * NT // NF):
                    for ds in range(NF):
                        ps = psp.tile([P, FD], F32, tag="mm")
                        mm_accum(
                            ps,
                            lambda k, ni=ni: ST_sb[:, k * N + ni * P: k * N + ni * P + P],
                            lambda k, ds=ds: V_sb[:, k * D + ds * FD: k * D + (ds + 1) * FD],
                        )
                        z_st = zstp.tile([P, FD], F32, tag="z")
                        nc.scalar.copy(z_st[:], ps[:])
                        nc.sync.dma_start(
                            out_z[ni * P:(ni + 1) * P, ds * FD:(ds + 1) * FD], z_st[:]
                        )

    nc.compile()
    return nc


@functools.cache
def _get_nc():
    return build_nc()


def _make_in_maps(x, Wq, Wk, Wv, Ws):
    bf = ml_dtypes.bfloat16
    idx = np.arange(N, dtype=np.float32)
    d2 = np.square(idx[:, None] - idx[None, :])  # exact in fp32
    w2 = (np.asarray(Wq, np.float32) @ np.asarray(Wk, np.float32).T).astype(bf)
    wv = np.asarray(Wv, np.float32).astype(bf)
    wsr = np.ascontiguousarray(
        np.asarray(Ws, np.float32)[:, 0].reshape(NT, P).T
    ).astype(bf)
    in_maps = []
    for b in range(B):
        xTb = np.ascontiguousarray(np.asarray(x[b], np.float32).T).astype(bf)
        in_maps.append(
            {"xT": xTb, "W2": w2, "Wv": wv, "Wsr": wsr, "d2": d2}
        )
    return in_maps


def run(x, Wq, Wk, Wv, Ws, trace=False):
    nc = _get_nc()
    in_maps = _make_in_maps(x, Wq, Wk, Wv, Ws)
    res = run_bass_kernel_spmd(nc, in_maps, core_ids=list(range(B)), trace=trace)
    Z = np.stack([res.results[b]["out_z"] for b in range(B)])
    Pp = np.stack([res.results[b]["out_p"] for b in range(B)])
    return (Z, Pp), res


def kernel(x, Wq, Wk, Wv, Ws):
    for _ in range(2):
        (Z, Pp), _ = run(x, Wq, Wk, Wv, Ws, trace=False)
        if np.isfinite(Z).all() and np.isfinite(Pp).all():
            break
    return Z, Pp
